# revision 4
# baseline (speedup 1.0000x reference)
"""GCNN (batched SpMM + GEMM + bias + ReLU) Trainium2 kernel — dense-stream.

Per-core work (one graph per NeuronCore, 8 graphs / 8 cores):
  out = relu(A @ (x @ W) + b),  A sparse [N, N] with E edges.

Key idea: per-edge gather/scatter DMA is descriptor-throughput-bound on
TRN2 (~6.8 ns/edge through the 4 SWDGE queues), so avoid indexed DMA
entirely.  Materialize A densely on the HOST (N=10000 -> 200 MB bf16 per
graph) and stream it through the PE as the *moving* matmul operand:

  phase 0: y_t = x_t @ W                  (79 tiles, y resident in SBUF bf16)
  main:    out^T[C, dst] accumulated in PSUM over src tiles t
           for each dst-supergroup S (<=2048 dst cols = 4 PSUM banks):
             for each src tile t: stream A panel [128 src, SW] bf16
               (4 panels per dma_start), matmuls with lhsT = y_t stationary,
               rhs = A slice (<=512 moving cols), accumulating into PSUM
             evict: relu(psum + b) on ACT (fused per-partition bias),
               DMA out^T slice

HBM traffic ~210 MB/core of pure sequential reads (no descriptors), PE
~420 us, wall ~ DMA-bound.  Output is computed transposed [C, N]; the
host transposes it back (free).

SPMD: one NEFF for all 8 cores; per-core data via input tensors.
"""

import sys

if "/opt/trn_rl_repo" not in sys.path:
    sys.path.insert(0, "/opt/trn_rl_repo")

import numpy as np
import ml_dtypes

import concourse.bacc as bacc
import concourse.mybir as mybir
from concourse import tile
from concourse.bass_utils import run_bass_kernel_spmd

BF16 = ml_dtypes.bfloat16

C = 128            # channels (C_IN == C_OUT == 128)
N = 10000          # nodes per graph
SRC_T = 79         # src tiles of 128 (last tile: 16 rows)
NPAD = SRC_T * 128          # 10112
SW = [2048, 2048, 2048, 2048, 1824]   # dst supergroup widths (sum = 10016)
NSG = len(SW)
DPAD = sum(SW)             # 10016
SG_OFF = np.cumsum([0] + SW).tolist()
# panel (S, t) column offset in the A stream
PAN_OFF = np.cumsum([0] + [SRC_T * w for w in SW]).tolist()
A_COLS = PAN_OFF[-1]       # 79 * 10016
TCHUNK = 8                 # src tiles per dma_start


# ---------------------------------------------------------------- host prep

def prep_core_inputs(x, edge_rows, edge_cols, edge_vals, W, b):
    """Build per-core input maps: dense bf16 A panel stream + transposed x."""
    Bn = x.shape[0]
    b_col = np.ascontiguousarray(b.astype(np.float32).reshape(C, 1))
    in_maps = []
    for g in range(Bn):
        A = np.zeros((NPAD, DPAD), dtype=np.float32)       # [src, dst]
        np.add.at(A, (np.asarray(edge_cols[g]), np.asarray(edge_rows[g])),
                  np.asarray(edge_vals[g]))
        Ab = A.astype(BF16)
        blocks = []
        for S in range(NSG):
            blk = Ab[:, SG_OFF[S]:SG_OFF[S + 1]]           # [NPAD, SW]
            blocks.append(np.ascontiguousarray(
                blk.reshape(SRC_T, 128, SW[S]).transpose(1, 0, 2)
            ).reshape(128, -1))
        Ar = np.ascontiguousarray(np.hstack(blocks))       # [128, A_COLS]
        in_maps.append({
            "xT": np.ascontiguousarray(x[g].T.astype(BF16)),
            "W": np.asarray(W).astype(BF16),
            "b_col": b_col,
            "A": Ar,
        })
    return in_maps


# ---------------------------------------------------------------- device IR

def build_nc():
    f32 = mybir.dt.float32
    bf16 = mybir.dt.bfloat16

    nc = bacc.Bacc("TRN2")
    xT_d = nc.dram_tensor("xT", [C, N], bf16, kind="ExternalInput")
    W_d = nc.dram_tensor("W", [C, C], bf16, kind="ExternalInput")
    bcol_d = nc.dram_tensor("b_col", [C, 1], f32, kind="ExternalInput")
    A_d = nc.dram_tensor("A", [128, A_COLS], bf16, kind="ExternalInput")
    outT_d = nc.dram_tensor("outT", [C, DPAD], bf16, kind="ExternalOutput")

    with tile.TileContext(nc) as tc:
        with (
            tc.tile_pool(name="const", bufs=1) as constp,
            tc.tile_pool(name="y", bufs=SRC_T) as ypool,
            tc.tile_pool(name="p0", bufs=3) as p0pool,
            tc.tile_pool(name="p0ps", bufs=2, space="PSUM") as p0ps,
            tc.tile_pool(name="a", bufs=3) as apool,
            tc.tile_pool(name="acc", bufs=6, space="PSUM") as accp,
            tc.tile_pool(name="ev", bufs=4) as evp,
        ):
            # ---- constants
            w_t = constp.tile([C, C], bf16, tag="w")
            nc.sync.dma_start(out=w_t[:], in_=W_d[:])
            bcol = constp.tile([C, 1], f32, tag="bcol")
            nc.sync.dma_start(out=bcol[:], in_=bcol_d[:])

            # ---- phase 0: y = x @ W resident in SBUF (bf16), one tile per t
            ytiles = []
            for t in range(SRC_T):
                rows = min(128, N - t * 128)
                yt = ypool.tile([128, C], bf16, tag="y", name=f"y_{t}")
                xt = p0pool.tile([C, 128], bf16, tag="xt")
                nc.sync.dma_start(out=xt[:, :rows],
                                  in_=xT_d[:, t * 128:t * 128 + rows])
                yps = p0ps.tile([128, C], f32, tag="yps")
                nc.tensor.matmul(yps[:rows, :], xt[:, :rows], w_t[:],
                                 start=True, stop=True)
                nc.vector.tensor_copy(yt[:rows, :], yps[:rows, :])
                ytiles.append(yt)

            # ---- main: stream dense A panels, accumulate out^T in PSUM
            for S in range(NSG):
                w = SW[S]
                nbank = (w + 511) // 512
                bw = [min(512, w - k * 512) for k in range(nbank)]
                ps = [accp.tile([128, 512], f32, tag="acc",
                                name=f"acc_{S}_{k}") for k in range(nbank)]
                last_rows = N - (SRC_T - 1) * 128
                for t0 in range(0, SRC_T, TCHUNK):
                    nt = min(TCHUNK, SRC_T - t0)
                    full = nt if t0 + nt < SRC_T else nt - 1
                    a_t = apool.tile([128, TCHUNK * 2048], bf16, tag="a")
                    off = PAN_OFF[S] + t0 * w
                    if full > 0:
                        nc.sync.dma_start(out=a_t[:, :full * w],
                                          in_=A_d[:, off:off + full * w])
                    if full < nt:   # trailing 16-row src tile
                        nc.sync.dma_start(
                            out=a_t[:last_rows, full * w:nt * w],
                            in_=A_d[:last_rows, off + full * w:off + nt * w])
                    for ti in range(nt):
                        t = t0 + ti
                        rows = 128 if t < SRC_T - 1 else last_rows
                        for k in range(nbank):
                            nc.tensor.matmul(
                                ps[k][:, :bw[k]],
                                ytiles[t][:rows, :],
                                a_t[:rows, ti * w + k * 512:ti * w + k * 512 + bw[k]],
                                start=(t == 0), stop=(t == SRC_T - 1))
                for k in range(nbank):
                    ot = evp.tile([128, 512], bf16, tag="ot")
                    nc.scalar.activation(
                        out=ot[:, :bw[k]], in_=ps[k][:, :bw[k]],
                        func=mybir.ActivationFunctionType.Relu,
                        bias=bcol[:])
                    col = SG_OFF[S] + k * 512
                    nc.sync.dma_start(out=outT_d[:, col:col + bw[k]],
                                      in_=ot[:, :bw[k]])

    nc.finalize()
    return nc


# ---------------------------------------------------------------- entry

def kernel(x, edge_rows, edge_cols, edge_vals, W, b):
    x = np.asarray(x, dtype=np.float32)
    W = np.asarray(W, dtype=np.float32)
    b = np.asarray(b, dtype=np.float32)

    Bn = x.shape[0]
    in_maps = prep_core_inputs(x, edge_rows, edge_cols, edge_vals, W, b)
    nc = build_nc()
    res = run_bass_kernel_spmd(nc, in_maps, list(range(Bn)))
    out = np.stack([
        np.asarray(r["outT"]).astype(np.float32)[:, :N].T for r in res.results
    ])
    return out
